# revision 15
# baseline (speedup 1.0000x reference)
"""EdgeNetwork GNN message-passing kernel for 8 Trainium2 NeuronCores.

Math (per edge e): mlp = relu(ef @ W1.T + b1)           [E,128]
                   A   = (mlp @ We).reshape(E,128,128)
                   msg[e] = A[e] @ x[range[e]]           [E,128]
                   out = segment_sum(msg, domain)        [N,128]

Strategy:
  * Host sorts edges by destination node and splits them into 8 contiguous
    node ranges with balanced edge counts -> each core owns a disjoint slice
    of output rows (no collective needed; host concatenates).
  * msg[e,h] = sum_{j,k} We[j, h*128+k] * mlp[e,j] * x[e,k]
    = one big matmul with stationary We' [(j,k), h] and moving
    P_T[(j,k), e] = mlp[e,j]*x[e,k]  (per-edge outer products).
  * P is built in natural layout [e,(j,k)] with one long free-dim
    tensor_tensor multiply using step-0 broadcast APs, then transposed to
    P_T with the DMA xbar transpose (bf16).
  * Scatter: edges are binned into 128-edge chunks whose destinations lie in
    a static 128-node window (window t covers local nodes [64t, 64t+128)).
    One-hot matmul per chunk -> [n_window, h] in PSUM, accumulated into an
    SBUF node accumulator at static offsets.
"""

import numpy as np
import ml_dtypes

import concourse.bacc as bacc
import concourse.mybir as mybir
import concourse.tile as tile
from concourse.bass_utils import run_bass_kernel_spmd
from concourse.masks import make_identity

H = 128
F = 64
N_NODES = 20000
N_EDGES = 32000
NCORES = 8
CHUNK = 128

BF16 = mybir.dt.bfloat16
F32 = mybir.dt.float32

_prog_cache = {}


# --------------------------------------------------------------------------
# host-side planning
# --------------------------------------------------------------------------

STRIDE = 64


def _assign_chunks(d_local, offsets):
    """Greedy assignment of sorted local dests to 128-edge chunks.

    ``offsets`` is a non-decreasing list of window starts (multiples of 64);
    chunk t accepts dests in [offsets[t], offsets[t]+128).
    Returns (slots, None) or (None, failed_window_offset).
    """
    offs = np.asarray(offsets)
    T = len(offs)
    fill = np.zeros(T, dtype=np.int64)
    slots = np.empty(len(d_local), dtype=np.int64)
    lo_all = np.searchsorted(offs, d_local - 127, side="left")
    hi_all = np.searchsorted(offs, d_local, side="right") - 1
    for i, d in enumerate(d_local):
        t = -1
        for cand in range(lo_all[i], hi_all[i] + 1):
            if fill[cand] < CHUNK:
                t = cand
                break
        if t < 0:
            return None, STRIDE * (int(d) // STRIDE)
        slots[i] = t * CHUNK + fill[t]
        fill[t] += 1
    return slots, None


def _plan(edge_domain):
    """Sort/split/window-assign. Returns dict with all static structure."""
    dest = np.asarray(edge_domain).astype(np.int64)
    order = np.argsort(dest, kind="stable")
    d_s = dest[order]

    # core boundaries: balanced edge counts snapped to node boundaries
    node_bounds = [0]
    for c in range(1, NCORES):
        nb = int(d_s[min((c * len(d_s)) // NCORES, len(d_s) - 1)])
        nb = max(nb, node_bounds[-1] + 1)
        node_bounds.append(nb)
    node_bounds.append(N_NODES)
    e_bounds = np.searchsorted(d_s, node_bounds)
    R = [node_bounds[c + 1] - node_bounds[c] for c in range(NCORES)]
    R_MAX = max(R)

    M = int(np.ceil(max(R_MAX - 128, 1) / STRIDE)) + 1
    mult = {STRIDE * i: 1 for i in range(M)}
    for _ in range(40):
        offsets = []
        for i in range(M):
            offsets += [STRIDE * i] * mult[STRIDE * i]
        if len(offsets) % 2:
            offsets.append(offsets[-1])
        slots_all = []
        bumps = set()
        for c in range(NCORES):
            dl = d_s[e_bounds[c]:e_bounds[c + 1]] - node_bounds[c]
            s, fail = _assign_chunks(dl, offsets)
            if s is None:
                bumps.add(min(fail, STRIDE * (M - 1)))
            else:
                slots_all.append(s)
        if not bumps:
            T = len(offsets)
            return dict(order=order, node_bounds=node_bounds,
                        e_bounds=e_bounds, slots=slots_all, T=T,
                        offsets=offsets, d_s=d_s,
                        NB=(offsets[-1] + 128 + 127) // 128)
        for b in bumps:
            mult[b] += 1
    raise RuntimeError("window assignment did not converge")


# --------------------------------------------------------------------------
# device program
# --------------------------------------------------------------------------

def _build_program(T, NB, offsets, loops=1):
    E_PAD = T * CHUNK
    G = T // 2                      # e-groups of 256

    nc = bacc.Bacc("TRN2", target_bir_lowering=False, debug=False,
                   num_devices=NCORES)

    efT = nc.dram_tensor("efT", [F + 1, E_PAD], BF16, kind="ExternalInput")
    xg = nc.dram_tensor("xg", [E_PAD, H], BF16, kind="ExternalInput")
    oneh = nc.dram_tensor("oneh", [E_PAD, 128], BF16, kind="ExternalInput")
    w1aT = nc.dram_tensor("w1aT", [F + 1, H], BF16, kind="ExternalInput")
    wep = nc.dram_tensor("wep", [128, H * H], BF16, kind="ExternalInput")
    out = nc.dram_tensor("out", [NB * 128, H], F32, kind="ExternalOutput")

    with tile.TileContext(nc) as tc:
        import contextlib

        with (
            tc.tile_pool(name="const", bufs=1) as constp,
            tc.tile_pool(name="acc", bufs=1) as accp,
            tc.tile_pool(name="small", bufs=4) as smallp,
            tc.tile_pool(name="pnat", bufs=3) as pnatp,
            tc.tile_pool(name="ptg", bufs=2) as ptgp,
            tc.tile_pool(name="msg", bufs=4) as msgp,
            tc.tile_pool(name="ps", bufs=2, space="PSUM") as psp,
            tc.tile_pool(name="ps2", bufs=2, space="PSUM") as ps2p,
            tc.For_i(0, loops, 1) if loops > 1 else contextlib.nullcontext(),
        ):
            # resident constants
            wep_sb = constp.tile([128, H * H], BF16)
            nc.sync.dma_start(out=wep_sb[:], in_=wep[:])
            w1aT_sb = constp.tile([F + 1, H], BF16)
            nc.sync.dma_start(out=w1aT_sb[:], in_=w1aT[:])
            ident = constp.tile([128, 128], F32)
            make_identity(nc, ident[:])

            nacc = accp.tile([128, NB * 128], F32)
            nc.vector.memset(nacc[:], 0.0)

            for g in range(G):
                ptg = ptgp.tile([128, 128, 256], BF16, tag="ptg")
                msgT_ps = psp.tile([128, 256], F32, tag="msgT")

                for half in range(2):
                    t = 2 * g + half
                    esl = slice(t * CHUNK, (t + 1) * CHUNK)

                    # ---- edge MLP ----
                    ef_t = smallp.tile([F + 1, 128], BF16, tag="ef")
                    nc.sync.dma_start(out=ef_t[:], in_=efT[:, esl])
                    mlp_ps = ps2p.tile([128, 128], F32, tag="mlp_ps")
                    nc.tensor.matmul(out=mlp_ps[:], lhsT=ef_t[:],
                                     rhs=w1aT_sb[:], start=True, stop=True)
                    mlp_t = smallp.tile([128, 128], BF16, tag="mlp")
                    nc.scalar.activation(mlp_t[:], mlp_ps[:],
                                         mybir.ActivationFunctionType.Relu)
                    # duplicate each mlp value into adjacent bf16 pairs so the
                    # P-build tensor_tensor qualifies for the DVE 2x_1p mode
                    # (every operand needs innermost step +/-1 of 2-byte elems)
                    mlpd = smallp.tile([128, 128, 2], BF16, tag="mlpd")
                    nc.vector.tensor_copy(
                        mlpd[:], mlp_t[:, :, None].to_broadcast([128, 128, 2]))

                    # ---- gathered source node features ----
                    x_t = smallp.tile([128, 128], BF16, tag="x")
                    nc.sync.dma_start(out=x_t[:], in_=xg[esl, :])

                    # ---- P build + transpose, in quarters ----
                    for q in range(4):
                        jsl = slice(q * 32, (q + 1) * 32)
                        pq = pnatp.tile([128, 32, 128], BF16, tag="pq")
                        in0 = x_t[:, None, :].to_broadcast([128, 32, 128])
                        # [e][j step 2][k64 step 0][pair step 1] — reads
                        # mlp[e,j] for both elements of each bf16 pair
                        in1 = mlpd[:, jsl, :][:, :, None, :].to_broadcast(
                            [128, 32, 64, 2])
                        nc.vector.tensor_tensor(out=pq[:], in0=in0, in1=in1,
                                                op=mybir.AluOpType.mult)
                        nc.sync.dma_start_transpose(
                            ptg[:, q * 32:(q + 1) * 32,
                                half * 128:(half + 1) * 128],
                            pq[:].rearrange("p a b -> p (a b)"))

                # ---- big contraction: msg_T[h, e] over (j,k) ----
                for c in range(128):
                    nc.tensor.matmul(out=msgT_ps[:],
                                     lhsT=wep_sb[:, c * 128:(c + 1) * 128],
                                     rhs=ptg[:, c, :],
                                     start=(c == 0), stop=(c == 127))

                msgT_sb = msgp.tile([128, 256], F32, tag="msgT_sb")
                nc.vector.tensor_copy(msgT_sb[:], msgT_ps[:])

                for half in range(2):
                    t = 2 * g + half
                    # transpose msg_T -> msg natural [e, h]
                    msg_ps = ps2p.tile([128, 128], F32, tag="msg_ps")
                    nc.tensor.transpose(
                        out=msg_ps[:],
                        in_=msgT_sb[:, half * 128:(half + 1) * 128],
                        identity=ident[:])
                    msg_sb = msgp.tile([128, 128], BF16, tag="msg_sb")
                    nc.vector.tensor_copy(msg_sb[:], msg_ps[:])

                    # one-hot scatter within the chunk's 128-node window
                    oh_t = smallp.tile([128, 128], BF16, tag="oh")
                    nc.sync.dma_start(out=oh_t[:],
                                      in_=oneh[t * CHUNK:(t + 1) * CHUNK, :])
                    sc_ps = ps2p.tile([128, 128], F32, tag="sc_ps")
                    nc.tensor.matmul(out=sc_ps[:], lhsT=oh_t[:],
                                     rhs=msg_sb[:], start=True, stop=True)

                    # accumulate into node buffer at this chunk's window
                    off = offsets[t]
                    p0 = off % 128
                    b0 = off // 128
                    n_lo = 128 - p0        # rows landing in block b0
                    csl0 = slice(b0 * 128, b0 * 128 + 128)
                    if p0 == 0:
                        nc.vector.tensor_add(
                            out=nacc[:, csl0], in0=nacc[:, csl0],
                            in1=sc_ps[:])
                    else:
                        csl1 = slice((b0 + 1) * 128, (b0 + 1) * 128 + 128)
                        nc.vector.tensor_add(
                            out=nacc[p0:128, csl0], in0=nacc[p0:128, csl0],
                            in1=sc_ps[0:n_lo, :])
                        nc.vector.tensor_add(
                            out=nacc[0:p0, csl1], in0=nacc[0:p0, csl1],
                            in1=sc_ps[n_lo:128, :])

            # ---- write out: out[b*128+p, h] = nacc[p, b*128+h] ----
            nacc_v = nacc[:].rearrange("p (b h) -> p b h", b=NB)
            out_v = out[:].rearrange("(b p) h -> p b h", b=NB)
            nc.sync.dma_start(out=out_v, in_=nacc_v)

    nc.compile()
    return nc


# --------------------------------------------------------------------------
# host-side data marshalling + run
# --------------------------------------------------------------------------

def _prep_inputs(plan, node_features, edge_features, edge_range, W1, b1, We):
    T = plan["T"]
    offsets = np.asarray(plan["offsets"])
    E_PAD = T * CHUNK
    order = plan["order"]
    e_bounds = plan["e_bounds"]
    node_bounds = plan["node_bounds"]

    ef_s = np.asarray(edge_features)[order]          # [E, F] sorted
    src_s = np.asarray(edge_range).astype(np.int64)[order]
    nf = np.asarray(node_features)

    w1aT = np.concatenate([np.asarray(W1).T, np.asarray(b1)[None, :]], axis=0)
    We3 = np.asarray(We).reshape(H, H, H)            # We[j, h, k]
    wep = np.ascontiguousarray(We3.transpose(2, 0, 1)).reshape(128, H * H)

    bf = ml_dtypes.bfloat16
    w1aT = w1aT.astype(bf)
    wep = wep.astype(bf)

    in_maps = []
    for c in range(NCORES):
        sl = slice(e_bounds[c], e_bounds[c + 1])
        slots = plan["slots"][c]
        d_local = plan["d_s"][sl] - node_bounds[c]

        efT = np.zeros((F + 1, E_PAD), dtype=bf)
        efT[:F, slots] = ef_s[sl].T.astype(bf)
        efT[F, slots] = 1.0

        xg = np.zeros((E_PAD, H), dtype=bf)
        xg[slots] = nf[src_s[sl]].astype(bf)

        oneh = np.zeros((E_PAD, 128), dtype=bf)
        t_of = slots // CHUNK
        oneh[slots, d_local - offsets[t_of]] = 1.0

        in_maps.append({"efT": efT, "xg": xg, "oneh": oneh,
                        "w1aT": w1aT, "wep": wep})
    return in_maps


def kernel(node_features, edge_features, edge_domain, edge_range,
           W1, b1, We):
    dest = np.asarray(edge_domain)
    plan = _plan(dest)

    key = (plan["T"], plan["NB"], tuple(plan["offsets"]))
    if key not in _prog_cache:
        _prog_cache[key] = _build_program(plan["T"], plan["NB"],
                                          plan["offsets"])
    nc = _prog_cache[key]

    in_maps = _prep_inputs(plan, node_features, edge_features, edge_range,
                           W1, b1, We)
    res = run_bass_kernel_spmd(nc, in_maps, list(range(NCORES)))

    out = np.zeros((N_NODES, H), dtype=np.float32)
    nb = plan["node_bounds"]
    for c in range(NCORES):
        r = nb[c + 1] - nb[c]
        out[nb[c]:nb[c + 1]] = res.results[c]["out"][:r]
    return out
